# revision 1
# baseline (speedup 1.0000x reference)
"""Trainium2 Bass kernel for nn_Attention_47545287967487.

Causal multi-head attention (B=2, S=2048, D=1024, H=16, DH=64) with QK
RMS-norm, distributed over 8 NeuronCores via head tensor-parallelism:
each core owns 2 heads (a 128-column slice of Wq/Wk/Wv and the matching
128-row slice of Wo), computes its partial output projection, and a
ReduceScatter produces each core's 512-row slice of the final output.

Numerics: projections and the output matmul run in float32r (~1e-4),
attention internals (QK^T, softmax, PV) run in bf16. Scores are bounded
(|q.k|/8 <= 8 after RMS-norm) so softmax skips the max-subtraction pass.

Engine plan per core:
 - PE: x@W projections (fp32r, K-tiled), QK^T with the two heads packed
   into array row-groups (tile_position), PV as [v|1]^T @ P so the
   softmax denominator is a free 65th output row, the output
   projection, selector matmuls for the per-head sum-of-squares
   reduction and rstd broadcast, and 128x128 transposes of v into
   [t,d] layout.
 - ACT: exp (softmax), Square, and rstd = exp(-0.5*ln(mean+eps)) --
   all functions live in one pinned ACT table so there are no table
   reloads. The 1/sqrt(DH) score scale is folded into q's rstd.
 - DVE: PSUM->SBUF casts/copies and the q/k normalize multiplies.
 - GPSIMD: partition-broadcast of the softmax denominator reciprocal
   and constant fills (otherwise idle).

The emission order software-pipelines chunks (x-prefetch, then the
previous chunk's attention, this chunk's projections, then the previous
chunk's output projection) so PE/ACT/DVE interleave across chunk
boundaries. build_nc(repeat=N) unrolls the whole pipeline N times in
one NEFF for slope-based device timing.

kernel(**inputs) takes the FULL unsharded inputs and returns the FULL
[2, 2048, 1024] float32 output.
"""

import math
import numpy as np

import concourse.bacc as bacc
import concourse.mybir as mybir
from concourse import tile
from concourse.bass_utils import run_bass_kernel_spmd

import ml_dtypes

BF16 = ml_dtypes.bfloat16

# Problem shape (hardcoded per the harness contract).
B, S, D, DH = 2, 2048, 1024, 64
H = D // DH
N_CORES = 8
HEADS_PER_CORE = H // N_CORES          # 2
DC = HEADS_PER_CORE * DH               # 128 feature columns per core
EPS = 1e-6

SCHUNK = 512                            # s-chunk width
TT = 128                                # t-tile width
KT = D // 128                           # 8 contraction tiles
NCH = S // SCHUNK                       # 4 s-chunks per batch
ROWS = B * S                            # 4096
ROWS_PER_CORE = ROWS // N_CORES         # 512

F32 = mybir.dt.float32
F32R = mybir.dt.float32r
BF = mybir.dt.bfloat16

# All ACT functions this kernel uses (Square, Ln, Exp, Copy) live in the
# 'natural_log_exp_and_others' table. The default table chooser picks the
# first table containing each function, which thrashes between the exp and
# ln tables (~1.3us per reload, dozens of reloads). Pin the chooser to the
# one table that covers everything by emptying the others (positions are
# preserved so act_func_set_id still indexes act_info.json correctly).
_PINNED_ACT_TABLE = "natural_log_exp_and_others"
_orig_get_act_tables = bacc.get_activation_tables


def _pinned_act_tables(arch):
    tables = _orig_get_act_tables(arch)
    return {
        name: (funcs if name == _PINNED_ACT_TABLE else set())
        for name, funcs in tables.items()
    }


bacc.get_activation_tables = _pinned_act_tables


def build_nc(collective=True, stage=3, repeat=1):
    nc = bacc.Bacc("TRN2", target_bir_lowering=False)

    xt_d = nc.dram_tensor("xt", [D, ROWS], F32R, kind="ExternalInput")
    wq_d = nc.dram_tensor("wq", [D, DC], F32R, kind="ExternalInput")
    wk_d = nc.dram_tensor("wk", [D, DC], F32R, kind="ExternalInput")
    wv_d = nc.dram_tensor("wv", [D, DC], F32R, kind="ExternalInput")
    wo_d = nc.dram_tensor("wo", [DC, D], F32R, kind="ExternalInput")
    mask_d = nc.dram_tensor("mask0", [TT, SCHUNK], BF, kind="ExternalInput")
    ident_d = nc.dram_tensor("ident", [128, 128], BF, kind="ExternalInput")
    ident32_d = nc.dram_tensor("ident32", [128, 128], F32, kind="ExternalInput")
    sel2_d = nc.dram_tensor("sel2", [128, 2], F32R, kind="ExternalInput")
    sel2t_d = nc.dram_tensor("sel2t", [2, 128], F32R, kind="ExternalInput")
    if collective:
        out_d = nc.dram_tensor("out", [ROWS_PER_CORE, D], F32, kind="ExternalOutput")
    else:
        # collective-free variant for TimelineSim: write partials straight out
        out_d = nc.dram_tensor("out", [ROWS, D], F32, kind="ExternalOutput")

    from contextlib import ExitStack
    with tile.TileContext(nc) as tc:
        with ExitStack() as ctx:
            consts = ctx.enter_context(tc.tile_pool(name="consts", bufs=1))
            wpool = ctx.enter_context(tc.tile_pool(name="wpool", bufs=1))
            persist = ctx.enter_context(tc.tile_pool(name="persist", bufs=1))
            xcp = ctx.enter_context(tc.tile_pool(name="xc", bufs=3))
            sqp = ctx.enter_context(tc.tile_pool(name="sqp", bufs=3))
            stdp = ctx.enter_context(tc.tile_pool(name="stdp", bufs=6))
            bcp = ctx.enter_context(tc.tile_pool(name="bcp", bufs=6))
            vtp = ctx.enter_context(tc.tile_pool(name="vtp", bufs=3))
            vaugp = ctx.enter_context(tc.tile_pool(name="vaugp", bufs=40))
            rkp = ctx.enter_context(tc.tile_pool(name="rkp", bufs=40))
            stgp = ctx.enter_context(tc.tile_pool(name="stgp", bufs=2))
            pp = ctx.enter_context(tc.tile_pool(name="pp", bufs=8))
            zbp = ctx.enter_context(tc.tile_pool(name="zbp", bufs=6))
            rcp = ctx.enter_context(tc.tile_pool(name="rcp", bufs=6))
            attallp = ctx.enter_context(tc.tile_pool(name="attall", bufs=3))
            outsbp = ctx.enter_context(tc.tile_pool(name="outsb", bufs=8))
            ps_acc = ctx.enter_context(tc.tile_pool(name="ps_acc", bufs=3, space="PSUM"))
            ps_pt = ctx.enter_context(tc.tile_pool(name="ps_pt", bufs=3, space="PSUM"))
            ps_att = ctx.enter_context(tc.tile_pool(name="ps_att", bufs=2, space="PSUM"))
            dram = ctx.enter_context(tc.tile_pool(name="dram", bufs=1, space="DRAM"))

            # ---- weights first (gate the first projections), then consts,
            # wo last (only needed at the first output projection) ----
            # One merged DMA per weight tensor: SBUF [128, KT*DC] where
            # free-column block k holds DRAM rows [128k, 128k+128) (the
            # k-th contraction tile), so lhsT slices stay [K=128, M=DC].
            w_sb = {}
            for wname, wd in (("q", wq_d), ("k", wk_d), ("v", wv_d)):
                t = wpool.tile([128, KT * DC], F32R, name=f"w{wname}")
                nc.sync.dma_start(
                    t[:].rearrange("p (k c) -> p k c", k=KT),
                    wd[:].rearrange("(k p) c -> p k c", p=128))
                for k in range(KT):
                    w_sb[(wname, k)] = t[:, k * DC:(k + 1) * DC]

            sel2_sb = consts.tile([128, 2], F32R, name="sel2_sb")
            nc.sync.dma_start(sel2_sb[:], sel2_d[:])
            sel2t_sb = consts.tile([2, 128], F32R, name="sel2t_sb")
            nc.sync.dma_start(sel2t_sb[:], sel2t_d[:])
            ident_sb = consts.tile([128, 128], BF, name="ident_sb")
            nc.sync.dma_start(ident_sb[:], ident_d[:])
            mask_sb = consts.tile([TT, SCHUNK], BF, name="mask_sb")
            nc.sync.dma_start(mask_sb[:], mask_d[:])
            ident32_sb = consts.tile([128, 128], F32, name="ident32_sb")
            nc.sync.dma_start(ident32_sb[:], ident32_d[:])
            eps_sb = consts.tile([128, 1], F32, name="eps_sb")
            nc.vector.memset(eps_sb[:], EPS)
            wo_sb = wpool.tile([DC, D], F32R, name="wo_sb")
            nc.sync.dma_start(wo_sb[:], wo_d[:])

            if collective:
                partial = dram.tile([ROWS, D], F32, name="partial")
                rs_out = dram.tile([ROWS_PER_CORE, D], F32, name="rs_out")
            else:
                partial = out_d
                rs_out = None

            # per-chunk q (normalized) / k (raw) bf16, feature-major.
            # Separate tiles per chunk so later-chunk writes never
            # false-share dependency tracking with earlier-chunk reads.
            qts = {}    # (b, i) -> [DC, SCHUNK] bf16
            kts = {}    # (b, i) -> [DC, SCHUNK] bf16
            vaug = {}   # (b, j) -> [128, 2*(DH+1)] bf16
            rk = {}     # (b, j) -> [128, 2] f32: rstd_k/8 per t-position

            def sumsq_rstd(acc_psum, b, i, tag, bias_ap):
                rep = rep_box[0]
                """Per-64-row-group mean-square -> rstd [2, 512] in SBUF.

                rstd = exp(-0.5 * ln(mean + eps) + bias_extra), with the
                optional extra bias (e.g. -ln 8) folded into the Exp's bias.
                """
                sq = sqp.tile([DC, SCHUNK], F32R, name=f"sq_{rep}_{tag}_{b}_{i}", tag="sq")
                nc.scalar.activation(sq[:], acc_psum[:],
                                     mybir.ActivationFunctionType.Square)
                sumsq = ps_pt.tile([2, SCHUNK], F32, name=f"ss_{rep}_{tag}_{b}_{i}",
                                   tag="pt")
                nc.tensor.matmul(sumsq[:], sel2_sb[:], sq[:], start=True, stop=True)
                lm = stdp.tile([2, SCHUNK], F32, name=f"lm_{rep}_{tag}_{b}_{i}", tag="std")
                nc.scalar.activation(lm[:], sumsq[:],
                                     mybir.ActivationFunctionType.Ln,
                                     scale=1.0 / DH, bias=eps_sb[:2, :])
                rstd = stdp.tile([2, SCHUNK], F32R, name=f"rstd_{rep}_{tag}_{b}_{i}",
                                 tag="rstd")
                nc.scalar.activation(rstd[:], lm[:],
                                     mybir.ActivationFunctionType.Exp,
                                     scale=-0.5, bias=bias_ap)
                return rstd

            ln8_sb = consts.tile([128, 1], F32, name="ln8_sb")
            nc.vector.memset(ln8_sb[:], -math.log(DH ** 0.5))

            xcs = {}
            at_alls = {}
            rep_box = [0]

            def prefetch_x(b, i):
                rep = rep_box[0]
                col0 = b * S + i * SCHUNK
                # ---- load xT chunk (one merged strided DMA) ----
                xc = xcp.tile([128, KT * SCHUNK], F32R, name=f"x_{rep}_{b}_{i}",
                              tag="xc")
                # one DMA per k-tile: cheaper first-tile latency, and the
                # first projection matmul can start before the rest land
                for k in range(KT):
                    nc.sync.dma_start(
                        xc[:, k * SCHUNK:(k + 1) * SCHUNK],
                        xt_d[k * 128:(k + 1) * 128, col0:col0 + SCHUNK])
                xcs[(b, i)] = xc

            def proj_q(b, i, xch):
                rep = rep_box[0]
                psq = ps_acc.tile([DC, SCHUNK], F32, name=f"pq_{rep}_{b}_{i}", tag="acc")
                for k in range(KT):
                    nc.tensor.matmul(psq[:], w_sb[("q", k)][:], xch[k][:],
                                     start=(k == 0), stop=(k == KT - 1))
                qtile = persist.tile([DC, SCHUNK], BF, name=f"qt_{rep}_{b}_{i}",
                                     tag="qtk", bufs=20)
                qts[(b, i)] = qtile
                rstd_q = sumsq_rstd(psq, b, i, "q", ln8_sb[:2, :])
                bcq = ps_pt.tile([DC, SCHUNK], F32, name=f"bcq_{rep}_{b}_{i}", tag="pt")
                nc.tensor.matmul(bcq[:], sel2t_sb[:], rstd_q[:],
                                 start=True, stop=True)
                bcqs = bcp.tile([DC, SCHUNK], F32, name=f"bcqs_{rep}_{b}_{i}", tag="bc")
                nc.vector.tensor_copy(bcqs[:], bcq[:])
                nc.vector.tensor_mul(qtile[:], psq[:], bcqs[:])

            def proj_k(b, i, xch):
                rep = rep_box[0]
                psk = ps_acc.tile([DC, SCHUNK], F32, name=f"pk_{rep}_{b}_{i}", tag="acc")
                for k in range(KT):
                    nc.tensor.matmul(psk[:], w_sb[("k", k)][:], xch[k][:],
                                     start=(k == 0), stop=(k == KT - 1))
                ktile = persist.tile([DC, SCHUNK], BF, name=f"kt_{rep}_{b}_{i}",
                                     tag="qtk", bufs=20)
                kts[(b, i)] = ktile
                rstd_k = sumsq_rstd(psk, b, i, "k", 0.0)
                bck = ps_pt.tile([DC, SCHUNK], F32, name=f"bck_{rep}_{b}_{i}", tag="pt")
                nc.tensor.matmul(bck[:], sel2t_sb[:], rstd_k[:],
                                 start=True, stop=True)
                bcks = bcp.tile([DC, SCHUNK], F32, name=f"bcks_{rep}_{b}_{i}", tag="bc")
                nc.vector.tensor_copy(bcks[:], bck[:])
                nc.vector.tensor_mul(ktile[:], psk[:], bcks[:])

            def proj_v(b, i, xch):
                rep = rep_box[0]
                psv = ps_acc.tile([DC, SCHUNK], F32, name=f"pv_{rep}_{b}_{i}", tag="acc")
                for k in range(KT):
                    nc.tensor.matmul(psv[:], w_sb[("v", k)][:], xch[k][:],
                                     start=(k == 0), stop=(k == KT - 1))
                vt = vtp.tile([DC, SCHUNK], BF, name=f"vt_{rep}_{b}_{i}", tag="vt")
                nc.vector.tensor_copy(vt[:], psv[:])
                for u in range(SCHUNK // TT):
                    j = i * (SCHUNK // TT) + u
                    tp = ps_pt.tile([128, 128], BF, name=f"tp_{rep}_{b}_{j}", tag="pt")
                    nc.tensor.transpose(tp[:], vt[:, u * 128:(u + 1) * 128],
                                        ident_sb[:])
                    va = vaugp.tile([128, 2 * (DH + 1)], BF,
                                    name=f"va_{rep}_{b}_{j}", tag="vaug")
                    nc.vector.tensor_copy(
                        va[:].rearrange("p (g d) -> p g d", g=2)[:, :, 0:DH],
                        tp[:].rearrange("p (g d) -> p g d", g=2))
                    nc.gpsimd.memset(
                        va[:].rearrange("p (g d) -> p g d", g=2)[:, :, DH:DH + 1],
                        1.0)
                    vaug[(b, j)] = va

            def proj_parts(b, i):
                xc = xcs.pop((b, i))
                xch = [xc[:, k * SCHUNK:(k + 1) * SCHUNK] for k in range(KT)]
                return [lambda: proj_q(b, i, xch),
                        lambda: proj_k(b, i, xch),
                        lambda: proj_v(b, i, xch)]

            def do_proj(b, i):
                for part in proj_parts(b, i):
                    part()

            def do_attn(b, i, weave=None):
                rep = rep_box[0]
                att = [ps_att.tile([DH + 1, SCHUNK], F32,
                                   name=f"att_{rep}_{b}_{i}_{h}", tag="att")
                       for h in range(HEADS_PER_CORE)]
                n_t = 4 * i + 4
                weave_at = {}
                if weave:
                    for w_idx, part in enumerate(weave):
                        weave_at[1 + w_idx * max(1, (n_t - 1) // len(weave))] = part
                for j in range(n_t):
                    if j in weave_at:
                        weave_at.pop(j)()
                    off = max(0, TT * (j - 4 * i))
                    npx = SCHUNK - off
                    jc, ju = j // 4, j % 4
                    pts = []
                    for h in range(HEADS_PER_CORE):
                        pt = ps_pt.tile([128, SCHUNK], F32,
                                        name=f"ptile_{rep}_{b}_{i}_{j}_{h}", tag="pt")
                        nc.tensor.matmul(
                            pt[:, :npx],
                            kts[(b, jc)][h * DH:(h + 1) * DH,
                                         ju * TT:(ju + 1) * TT],
                            qts[(b, i)][h * DH:(h + 1) * DH, off:SCHUNK],
                            start=True, stop=True,
                            tile_position=(h * DH, 0),
                        )
                        pts.append(pt)
                    for h in range(HEADS_PER_CORE):
                        psb = pp.tile([128, SCHUNK], BF,
                                      name=f"p_{rep}_{b}_{i}_{j}_{h}", tag="p")
                        nc.scalar.activation(psb[:, :npx], pts[h][:, :npx],
                                             mybir.ActivationFunctionType.Exp)
                        if j >= 4 * i:
                            nc.vector.tensor_mul(psb[:, :npx], psb[:, :npx],
                                                 mask_sb[:, :npx])
                        nc.tensor.matmul(
                            att[h][:, off:SCHUNK],
                            vaug[(b, j)][:, h * (DH + 1):(h + 1) * (DH + 1)],
                            psb[:, :npx],
                            start=(j == 0), stop=(j == n_t - 1),
                        )

                for part in list(weave_at.values()):
                    part()

                # ---- normalize by softmax denominator ----
                at_all = attallp.tile([DC, SCHUNK], F32R,
                                      name=f"atall_{rep}_{b}_{i}", tag="attall")
                for h in range(HEADS_PER_CORE):
                    rc = rcp.tile([1, SCHUNK], F32, name=f"rc_{rep}_{b}_{i}_{h}",
                                  tag="rc")
                    nc.vector.reciprocal(rc[:], att[h][DH:DH + 1, :])
                    zbs = zbp.tile([DH, SCHUNK], F32, name=f"zbs_{rep}_{b}_{i}_{h}",
                                   tag="zb")
                    nc.gpsimd.partition_broadcast(zbs[:], rc[:])
                    nc.vector.tensor_mul(at_all[h * DH:(h + 1) * DH, :],
                                         att[h][0:DH, :], zbs[:])

                at_alls[(b, i)] = at_all

            def final_u(b, i, u, at_all):
                rep = rep_box[0]
                # ---- partial output projection (fp32r), one 128-row slab ----
                for n in range(D // 512):
                    op = ps_pt.tile([128, 512], F32,
                                    name=f"op_{rep}_{b}_{i}_{u}_{n}", tag="pt")
                    nc.tensor.matmul(op[:],
                                     at_all[:, u * 128:(u + 1) * 128],
                                     wo_sb[:, n * 512:(n + 1) * 512],
                                     start=True, stop=True)
                    osb = outsbp.tile([128, 512], F32,
                                      name=f"osb_{rep}_{b}_{i}_{u}_{n}",
                                      tag="outsb")
                    nc.vector.tensor_copy(osb[:], op[:])
                    r0 = b * S + i * SCHUNK + u * 128
                    nc.sync.dma_start(
                        partial[r0:r0 + 128, n * 512:(n + 1) * 512],
                        osb[:])

            def final_parts(b, i):
                at_all = at_alls.pop((b, i))
                return [(lambda u=u: final_u(b, i, u, at_all))
                        for u in range(SCHUNK // 128)]

            def do_final(b, i):
                for part in final_parts(b, i):
                    part()


            # Software pipeline. Per step: prefetch x for chunk ci (so its
            # DMA issues ahead of the previous chunk's output stores),
            # attention for chunk ci-1 (its inputs are ready, filling PE/ACT
            # while the x DMA streams), then projections for chunk ci.
            chunks = [(b, i) for b in range(B) for i in range(NCH)]
            for rep_i in range(repeat):
                rep_box[0] = rep_i
                fparts = {}
                for ci in range(len(chunks) + 1):
                    if ci == 0:
                        prefetch_x(*chunks[0])
                        prefetch_x(*chunks[1])
                    elif ci + 1 < len(chunks):
                        prefetch_x(*chunks[ci + 1])
                    if ci >= 1 and stage >= 2:
                        do_attn(*chunks[ci - 1],
                                weave=fparts.pop(ci - 2, None))
                    if ci < len(chunks):
                        do_proj(*chunks[ci])
                    if ci >= 1 and stage >= 3:
                        fparts[ci - 1] = final_parts(*chunks[ci - 1])
                for parts in fparts.values():
                    for part in parts:
                        part()
            if stage == 1:
                # flush qt/kt so the pipeline isn't dead code
                for (b, i), t in qts.items():
                    r0 = b * S + i * SCHUNK
                    nc.sync.dma_start(
                        partial[r0:r0 + 128, 0:256],
                        t[0:128, :].bitcast(F32))
                for (b, i), t in kts.items():
                    r0 = b * S + i * SCHUNK
                    nc.sync.dma_start(
                        partial[r0:r0 + 128, 256:512],
                        t[0:128, :].bitcast(F32))
            elif stage == 2:
                for (b, i), t in list(at_alls.items()):
                    r0 = b * S + i * SCHUNK
                    nc.sync.dma_start(partial[r0:r0 + 128, 0:512],
                                      t[:].bitcast(F32))
            # ---- ReduceScatter partial outputs across the 8 cores ----
            if collective:
                nc.gpsimd.collective_compute(
                    "ReduceScatter",
                    mybir.AluOpType.add,
                    replica_groups=[list(range(N_CORES))],
                    ins=[partial[:]],
                    outs=[rs_out[:]],
                )
                nc.sync.dma_start(out_d[:], rs_out[:])

    nc.compile()
    return nc


_NC_CACHE = {}


def _get_nc():
    if "nc" not in _NC_CACHE:
        _NC_CACHE["nc"] = build_nc()
    return _NC_CACHE["nc"]


def _host_inputs(x, Wq, Wk, Wv, Wo):
    xt = np.ascontiguousarray(x.reshape(ROWS, D).T).astype(np.float32)
    mask0 = (np.arange(TT)[:, None] <= np.arange(SCHUNK)[None, :]).astype(BF16)
    ident = np.eye(128, dtype=BF16)
    ident32 = np.eye(128, dtype=np.float32)
    sel2 = np.zeros((128, 2), dtype=np.float32)
    sel2[:DH, 0] = 1.0
    sel2[DH:2 * DH, 1] = 1.0
    sel2t = np.ascontiguousarray(sel2.T)

    in_maps = []
    for c in range(N_CORES):
        cs = c * DC
        in_maps.append({
            "xt": xt,
            "wq": np.ascontiguousarray(Wq[:, cs:cs + DC]).astype(np.float32),
            "wk": np.ascontiguousarray(Wk[:, cs:cs + DC]).astype(np.float32),
            "wv": np.ascontiguousarray(Wv[:, cs:cs + DC]).astype(np.float32),
            "wo": np.ascontiguousarray(Wo[cs:cs + DC, :]).astype(np.float32),
            "mask0": mask0,
            "ident": ident,
            "ident32": ident32,
            "sel2": sel2,
            "sel2t": sel2t,
        })
    return in_maps


def kernel(x, Wq, Wk, Wv, Wo, mask):
    x = np.asarray(x, dtype=np.float32)
    nc = _get_nc()
    in_maps = _host_inputs(x, np.asarray(Wq), np.asarray(Wk),
                           np.asarray(Wv), np.asarray(Wo))
    res = run_bass_kernel_spmd(nc, in_maps, list(range(N_CORES)))
    full = np.concatenate([res.results[c]["out"] for c in range(N_CORES)], axis=0)
    return full.reshape(B, S, D)


if __name__ == "__main__":
    nc = build_nc()
    print("kernel built and compiled OK")



# revision 23
# speedup vs baseline: 1.4377x; 1.4377x over previous
"""Trainium2 Bass kernel for nn_Attention_47545287967487.

Causal multi-head attention (B=2, S=2048, D=1024, H=16, DH=64) with QK
RMS-norm, distributed over 8 NeuronCores.

Distribution: head tensor-parallel for the QKV projections and attention
(each core owns 2 heads = a 128-column slice of Wq/Wk/Wv, computing the
full 4096-row sequence), then per-batch AllToAlls redistribute the bf16
attention outputs so each core owns a 256-row slice per batch and runs
the output projection locally with the full Wo. This moves ~1MB/core
over the fabric instead of ReduceScattering a 16MB fp32 partial, and
the final out write is 2MB instead of 16MB. The batch-0 AllToAll
overlaps batch-1 attention; its output projection weaves into the
batch-1 pipeline.

Numerics: x/W/QK^T/PV run in bf16 (fp32 PSUM accumulation), softmax in
fp32->bf16. Scores are bounded (|q.k|/8 <= 8 after RMS-norm) so softmax
skips the max-subtraction pass; a constant -2.25 bias inside the exp
keeps headroom for an fp8 probability variant (cancels in the
normalization).

Engine plan per core:
 - PE: x@W projections (bf16, K-tiled), QK^T with the two heads packed
   into array row-groups (tile_position), PV as [v|1]^T @ P so the
   softmax denominator is a free 65th output row, the local output
   projection, selector matmuls for the q+k sum-of-squares rows and the
   rstd broadcast, and 128x128 transposes of v into [t,d] layout.
 - ACT: one exp per (chunk, key-tile) covering BOTH heads via a
   [128,2,npx] access pattern over a 2-bank PSUM pair tile; ONE
   Square / Ln / Exp per chunk for the q+k rstd chain (q and k side by
   side on partitions 0:2 of a [2,1024] tile). The 1/sqrt(DH) score
   scale folds into the q normalize multiply.
 - DVE: PSUM->SBUF casts/copies, q/k normalize multiplies, causal mask
   multiplies (only the 128-col diagonal block), softmax denominator
   reciprocals and attention-output normalize multiplies.
 - Pool/GPSIMD: denominator partition broadcasts and the AllToAll
   collectives (everything else is kept off Pool so a collective's
   engine-occupancy can't stall the compute pipeline; att accumulators
   are copied PSUM->SBUF so their PSUM slots recycle without waiting on
   the normalize chain).

PSUM discipline: exactly 8 banks = one pool with a [128,1024] fp32
"pair" tag (bufs=3; holds q+k projection pairs, v projections + v
transposes, the sumsq/broadcast scratch, score pairs, and output-
projection pairs) plus two [65,512] attention accumulators.

kernel(**inputs) takes the FULL unsharded inputs and returns the FULL
[2, 2048, 1024] float32 output.
"""

import math
import numpy as np

import concourse.bacc as bacc
import concourse.mybir as mybir
from concourse import tile
from concourse.bass_utils import run_bass_kernel_spmd

import ml_dtypes

BF16 = ml_dtypes.bfloat16

# Problem shape (hardcoded per the harness contract).
B, S, D, DH = 2, 2048, 1024, 64
H = D // DH
N_CORES = 8
HEADS_PER_CORE = H // N_CORES          # 2
DC = HEADS_PER_CORE * DH               # 128 feature columns per core
EPS = 1e-6

SCHUNK = 512                            # s-chunk width
TT = 128                                # t-tile width
KT = D // 128                           # 8 contraction tiles
NCH = S // SCHUNK                       # 4 s-chunks per batch
ROWS = B * S                            # 4096
RPB = S // N_CORES                      # 256 output rows per core per batch
EXP_BIAS = -2.25                        # softmax headroom shift (cancels)
QSCALE = 1.0 / (DH ** 0.5)              # folded into the q normalize

F32 = mybir.dt.float32
F32R = mybir.dt.float32r
BF = mybir.dt.bfloat16

# All ACT functions this kernel uses (Square, Ln, Exp, Copy) live in the
# 'natural_log_exp_and_others' table. The default table chooser picks the
# first table containing each function, which thrashes between the exp and
# ln tables (~1.3us per reload, dozens of reloads). Pin the chooser to the
# one table that covers everything by emptying the others (positions are
# preserved so act_func_set_id still indexes act_info.json correctly).
_PINNED_ACT_TABLE = "natural_log_exp_and_others"
_orig_get_act_tables = bacc.get_activation_tables


def _pinned_act_tables(arch):
    tables = _orig_get_act_tables(arch)
    return {
        name: (funcs if name == _PINNED_ACT_TABLE else set())
        for name, funcs in tables.items()
    }


bacc.get_activation_tables = _pinned_act_tables


def build_nc(collective=True, stage=3, repeat=1):
    nc = bacc.Bacc("TRN2", target_bir_lowering=False)

    xt_d = nc.dram_tensor("xt", [D, ROWS], BF, kind="ExternalInput")
    wq_d = nc.dram_tensor("wq", [D, DC], BF, kind="ExternalInput")
    wk_d = nc.dram_tensor("wk", [D, DC], BF, kind="ExternalInput")
    wv_d = nc.dram_tensor("wv", [D, DC], BF, kind="ExternalInput")
    wo_d = nc.dram_tensor("wo", [D, D], BF, kind="ExternalInput")
    mask2_d = nc.dram_tensor("mask2", [TT, 2 * TT], BF, kind="ExternalInput")
    ident_d = nc.dram_tensor("ident", [128, 128], BF, kind="ExternalInput")
    sel2_d = nc.dram_tensor("sel2", [128, 2], BF, kind="ExternalInput")
    sel2t_d = nc.dram_tensor("sel2t", [2, 128], BF, kind="ExternalInput")
    out_d = nc.dram_tensor("out", [2 * RPB, D], F32, kind="ExternalOutput")

    from contextlib import ExitStack
    with tile.TileContext(nc) as tc:
        with ExitStack() as ctx:
            consts = ctx.enter_context(tc.tile_pool(name="consts", bufs=1))
            wpool = ctx.enter_context(tc.tile_pool(name="wpool", bufs=1))
            persist = ctx.enter_context(tc.tile_pool(name="persist", bufs=1))
            xcp = ctx.enter_context(tc.tile_pool(name="xc", bufs=2))
            sqp = ctx.enter_context(tc.tile_pool(name="sqp", bufs=2))
            stdp = ctx.enter_context(tc.tile_pool(name="stdp", bufs=2))
            bcp = ctx.enter_context(tc.tile_pool(name="bcp", bufs=2))
            vtp = ctx.enter_context(tc.tile_pool(name="vtp", bufs=2))
            vaugp = ctx.enter_context(tc.tile_pool(name="vaugp", bufs=36))
            pp = ctx.enter_context(tc.tile_pool(name="pp", bufs=4))
            attsbp = ctx.enter_context(tc.tile_pool(name="attsb", bufs=4))
            zbp = ctx.enter_context(tc.tile_pool(name="zbp", bufs=4))
            rcp = ctx.enter_context(tc.tile_pool(name="rcp", bufs=4))
            attallp = ctx.enter_context(tc.tile_pool(name="attall", bufs=3))
            gsp = ctx.enter_context(tc.tile_pool(name="gsp", bufs=2))
            outsbp = ctx.enter_context(tc.tile_pool(name="outsb", bufs=2))
            ps = ctx.enter_context(tc.tile_pool(name="ps", bufs=1, space="PSUM"))
            dram = ctx.enter_context(tc.tile_pool(name="dram", bufs=1, space="DRAM"))

            # ---- weights first (gate the first projections), then consts,
            # wo last (only needed at the first output projection) ----
            # One merged DMA per weight tensor: SBUF [128, KT*DC] where
            # free-column block k holds DRAM rows [128k, 128k+128) (the
            # k-th contraction tile), so lhsT slices stay [K=128, M=DC].
            w_sb = {}
            for wname, wd in (("q", wq_d), ("k", wk_d), ("v", wv_d)):
                t = wpool.tile([128, KT * DC], BF, name=f"w{wname}")
                nc.sync.dma_start(
                    t[:].rearrange("p (k c) -> p k c", k=KT),
                    wd[:].rearrange("(k p) c -> p k c", p=128))
                for k in range(KT):
                    w_sb[(wname, k)] = t[:, k * DC:(k + 1) * DC]

            sel2_sb = consts.tile([128, 2], BF, name="sel2_sb")
            nc.sync.dma_start(sel2_sb[:], sel2_d[:])
            sel2t_sb = consts.tile([2, 128], BF, name="sel2t_sb")
            nc.sync.dma_start(sel2t_sb[:], sel2t_d[:])
            ident_sb = consts.tile([128, 128], BF, name="ident_sb")
            nc.sync.dma_start(ident_sb[:], ident_d[:])
            mask2_sb = consts.tile([TT, 2 * TT], BF, name="mask2_sb")
            nc.sync.dma_start(mask2_sb[:], mask2_d[:])
            eps_sb = consts.tile([2, 1], F32, name="eps_sb")
            nc.vector.memset(eps_sb[:], EPS)
            zero_sb = consts.tile([2, 1], F32, name="zero_sb")
            nc.vector.memset(zero_sb[:], 0.0)
            ebias_sb = consts.tile([128, 1], F32, name="ebias_sb")
            nc.vector.memset(ebias_sb[:], EXP_BIAS)
            # wo is only needed by the first output projection (~half-way in);
            # its 2MB DMA is deferred into the pipeline so it doesn't delay
            # the first x-chunk prefetches behind it in the queue.
            wo_sb = wpool.tile([128, KT * D], BF, name="wo_sb")
            wo_loaded = [False]

            def load_wo():
                if not wo_loaded[0]:
                    wo_loaded[0] = True
                    nc.sync.dma_start(
                        wo_sb[:].rearrange("p (k c) -> p k c", k=KT),
                        wo_d[:].rearrange("(k p) c -> p k c", p=128))

            # DRAM staging for the per-batch AllToAll of attention outputs.
            # a[b] row-block j = this core's 128 features for row-range
            # [RPB*j, RPB*j+RPB) of batch b; after AllToAll g[b] row-block s
            # = core s's features for THIS core's row range.
            a_dr = [dram.tile([N_CORES * DC, RPB], BF, name=f"a_dr{b}")
                    for b in range(B)]
            if collective:
                g_dr = [dram.tile([N_CORES * DC, RPB], BF, name=f"g_dr{b}")
                        for b in range(B)]
            else:
                g_dr = a_dr  # collective-free variant for TimelineSim

            # per-chunk normalized q/k bf16, feature-major.
            qts = {}    # (b, i) -> [DC, SCHUNK] bf16, q * rstd_q / sqrt(DH)
            kts = {}    # (b, i) -> [DC, SCHUNK] bf16, k * rstd_k
            vaug = {}   # (b, j) -> [128, 2*(DH+1)] bf16: [v|1] per head
            pqks = {}   # (b, i) -> in-flight q|k projection PSUM pair

            xcs = {}
            rep_box = [0]

            def prefetch_x(b, i):
                rep = rep_box[0]
                col0 = b * S + i * SCHUNK
                xc = xcp.tile([128, KT * SCHUNK], BF, name=f"x_{rep}_{b}_{i}",
                              tag="xc")
                # one DMA per k-tile: cheaper first-tile latency, and the
                # first projection matmul can start before the rest land
                for k in range(KT):
                    nc.sync.dma_start(
                        xc[:, k * SCHUNK:(k + 1) * SCHUNK],
                        xt_d[k * 128:(k + 1) * 128, col0:col0 + SCHUNK])
                xcs[(b, i)] = xc

            def proj_qk_mm(b, i, xch, half, ks):
                rep = rep_box[0]
                if half == 0 and ks[0] == 0:
                    pqks[(b, i)] = ps.tile(
                        [128, 2 * SCHUNK], F32, name=f"pqk_{rep}_{b}_{i}",
                        tag="pair", bufs=3)
                pqk = pqks[(b, i)]
                wname = "qk"[half]
                for k in ks:
                    nc.tensor.matmul(
                        pqk[:, half * SCHUNK:(half + 1) * SCHUNK],
                        w_sb[(wname, k)][:], xch[k][:], start=(k == 0),
                        stop=(k == KT - 1))

            def proj_qk_norm(b, i):
                rep = rep_box[0]
                # raw q|k off PSUM immediately (frees the pair slot); the
                # whole normalize chain then runs in bf16 (2x DVE/ACT).
                pqk = pqks.pop((b, i))
                qkr = sqp.tile([128, 2 * SCHUNK], BF, name=f"qkr_{rep}_{b}_{i}",
                               tag="qkr")
                nc.vector.tensor_copy(qkr[:], pqk[:])
                sq = sqp.tile([128, 2 * SCHUNK], BF, name=f"sq_{rep}_{b}_{i}",
                              tag="sq")
                nc.scalar.activation(sq[:], qkr[:],
                                     mybir.ActivationFunctionType.Square)

                # sum-of-squares per 64-row head group: q heads at cols
                # [0:512] of rows 0:2, k heads at cols [512:1024].
                ssbc = ps.tile([128, 2 * SCHUNK], F32, name=f"ssbc_{rep}_{b}_{i}",
                               tag="pair", bufs=3)
                nc.tensor.matmul(ssbc[0:2, 0:SCHUNK], sel2_sb[:],
                                 sq[:, 0:SCHUNK], start=True, stop=True)
                nc.tensor.matmul(ssbc[0:2, SCHUNK:2 * SCHUNK], sel2_sb[:],
                                 sq[:, SCHUNK:2 * SCHUNK], start=True,
                                 stop=True)
                lm = stdp.tile([2, 2 * SCHUNK], F32, name=f"lm_{rep}_{b}_{i}",
                               tag="lm")
                nc.scalar.activation(lm[:], ssbc[0:2, :],
                                     mybir.ActivationFunctionType.Ln,
                                     scale=1.0 / DH, bias=eps_sb[:])
                rstd = stdp.tile([2, 2 * SCHUNK], BF, name=f"rstd_{rep}_{b}_{i}",
                                 tag="rstd")
                nc.scalar.activation(rstd[:], lm[:],
                                     mybir.ActivationFunctionType.Exp,
                                     scale=-0.5, bias=zero_sb[:])

                # broadcast both rstds over the head 64-row groups (WAR with
                # the sumsq rows is tracked; Ln has consumed them by then)
                for half in range(2):
                    nc.tensor.matmul(
                        ssbc[:, half * SCHUNK:(half + 1) * SCHUNK],
                        sel2t_sb[:], rstd[:, half * SCHUNK:(half + 1) * SCHUNK],
                        start=True, stop=True)
                bcs = bcp.tile([DC, 2 * SCHUNK], BF, name=f"bcs_{rep}_{b}_{i}",
                               tag="bc")
                nc.vector.tensor_copy(bcs[:], ssbc[:])

                qtile = persist.tile([DC, SCHUNK], BF, name=f"qt_{rep}_{b}_{i}",
                                     tag="qt", bufs=3)
                qts[(b, i)] = qtile
                nc.vector.scalar_tensor_tensor(
                    qtile[:], qkr[:, 0:SCHUNK], QSCALE, bcs[:, 0:SCHUNK],
                    mybir.AluOpType.mult, mybir.AluOpType.mult)
                ktile = persist.tile([DC, SCHUNK], BF, name=f"kt_{rep}_{b}_{i}",
                                     tag="kt", bufs=8)
                kts[(b, i)] = ktile
                nc.vector.tensor_mul(ktile[:], qkr[:, SCHUNK:2 * SCHUNK],
                                     bcs[:, SCHUNK:2 * SCHUNK])

            def proj_v_mm(b, i, xch, ks):
                rep = rep_box[0]
                if ks[0] == 0:
                    pqks[(b, i, "v")] = ps.tile(
                        [128, 2 * SCHUNK], F32, name=f"pv_{rep}_{b}_{i}",
                        tag="pair", bufs=3)
                psv = pqks[(b, i, "v")]
                for k in ks:
                    nc.tensor.matmul(psv[:, 0:SCHUNK], w_sb[("v", k)][:],
                                     xch[k][:], start=(k == 0),
                                     stop=(k == KT - 1))

            def proj_v_fin(b, i):
                rep = rep_box[0]
                psv = pqks.pop((b, i, "v"))
                vt = vtp.tile([DC, SCHUNK], BF, name=f"vt_{rep}_{b}_{i}",
                              tag="vt")
                nc.vector.tensor_copy(vt[:], psv[:, 0:SCHUNK])
                # transposes reuse the (dead) second bank of the psv slot
                for u in range(SCHUNK // TT):
                    j = i * (SCHUNK // TT) + u
                    tpb = psv[:, SCHUNK + 64 * u:SCHUNK + 64 * (u + 1)].bitcast(BF)
                    nc.tensor.transpose(tpb[:], vt[:, u * 128:(u + 1) * 128],
                                        ident_sb[:])
                    va = vaugp.tile([128, 2 * (DH + 1)], BF,
                                    name=f"va_{rep}_{b}_{j}", tag="vaug")
                    nc.vector.tensor_copy(
                        va[:].rearrange("p (g d) -> p g d", g=2)[:, :, 0:DH],
                        tpb[:].rearrange("p (g d) -> p g d", g=2))
                    nc.vector.memset(
                        va[:].rearrange("p (g d) -> p g d", g=2)[:, :, DH:DH + 1],
                        1.0)
                    vaug[(b, j)] = va

            def proj_parts(b, i):
                xc = xcs.pop((b, i))
                xch = [xc[:, k * SCHUNK:(k + 1) * SCHUNK] for k in range(KT)]
                return [
                    lambda: proj_qk_mm(b, i, xch, 0, [0, 1, 2, 3]),
                    lambda: proj_qk_mm(b, i, xch, 0, [4, 5, 6, 7]),
                    lambda: proj_qk_mm(b, i, xch, 1, [0, 1, 2, 3]),
                    lambda: proj_qk_mm(b, i, xch, 1, [4, 5, 6, 7]),
                    lambda: proj_qk_norm(b, i),
                    lambda: proj_v_mm(b, i, xch, [0, 1, 2, 3]),
                    lambda: proj_v_mm(b, i, xch, [4, 5, 6, 7]),
                    lambda: proj_v_fin(b, i),
                ]

            def do_proj(b, i):
                for part in proj_parts(b, i):
                    part()

            def do_attn(b, i, weave=None):
                rep = rep_box[0]
                att = [ps.tile([DH + 1, SCHUNK], F32,
                               name=f"att_{rep}_{b}_{i}_{h}", tag=f"att{h}",
                               bufs=1)
                       for h in range(HEADS_PER_CORE)]
                n_t = 4 * i + 4
                parts = list(weave) if weave else []
                npop = len(parts)
                for j in range(n_t):
                    off = max(0, TT * (j - 4 * i))
                    npx = SCHUNK - off
                    jc, ju = j // 4, j % 4
                    # both heads' scores in one 2-bank pair tile: head h at
                    # cols [512h+off, 512h+512)
                    pt = ps.tile([128, 2 * SCHUNK], F32,
                                 name=f"ptile_{rep}_{b}_{i}_{j}", tag="pair", bufs=3)
                    for h in range(HEADS_PER_CORE):
                        nc.tensor.matmul(
                            pt[:, SCHUNK * h + off:SCHUNK * (h + 1)],
                            kts[(b, jc)][h * DH:(h + 1) * DH,
                                         ju * TT:(ju + 1) * TT],
                            qts[(b, i)][h * DH:(h + 1) * DH, off:SCHUNK],
                            start=True, stop=True,
                            tile_position=(h * DH, 0),
                        )
                    # one exp covers both heads via the [128, 2, npx] view
                    psb = pp.tile([128, 2 * SCHUNK], BF,
                                  name=f"p_{rep}_{b}_{i}_{j}", tag="p")
                    ptv = pt[:].rearrange("p (h c) -> p h c", h=2)
                    psv = psb[:].rearrange("p (h c) -> p h c", h=2)
                    nc.scalar.activation(
                        psv[:, :, off:SCHUNK], ptv[:, :, off:SCHUNK],
                        mybir.ActivationFunctionType.Exp, bias=ebias_sb[:])
                    if j >= 4 * i:
                        # causal mask: only the first 128 columns of the
                        # diagonal block differ from all-ones
                        nc.vector.tensor_mul(
                            psv[:, :, off:off + TT], psv[:, :, off:off + TT],
                            mask2_sb[:].rearrange("p (h c) -> p h c", h=2))
                    # weave projection/output-projection work between the exp
                    # and the PV matmuls, paced evenly across ALL j-tiles so
                    # the PE stays fed while ACT runs the exp chain
                    want = (npop * (j + 1)) // n_t
                    while npop - len(parts) < want:
                        parts.pop(0)()
                    for h in range(HEADS_PER_CORE):
                        nc.tensor.matmul(
                            att[h][:, off:SCHUNK],
                            vaug[(b, j)][:, h * (DH + 1):(h + 1) * (DH + 1)],
                            psb[:, SCHUNK * h + off:SCHUNK * (h + 1)],
                            start=(j == 0), stop=(j == n_t - 1),
                        )

                for part in parts:
                    part()

                # copy accumulators off PSUM immediately (releases the att
                # banks for the next chunk without waiting on the normalize
                # chain, which can stall behind a collective on Pool)
                asb = attsbp.tile([DH + 1, 2 * SCHUNK], F32,
                                  name=f"asb_{rep}_{b}_{i}", tag="asb")
                for h in range(HEADS_PER_CORE):
                    nc.vector.tensor_copy(
                        asb[:, h * SCHUNK:(h + 1) * SCHUNK], att[h][:])

                # ---- normalize by softmax denominator ----
                at_all = attallp.tile([DC, SCHUNK], BF,
                                      name=f"atall_{rep}_{b}_{i}", tag="attall")
                for h in range(HEADS_PER_CORE):
                    rc = rcp.tile([1, SCHUNK], F32, name=f"rc_{rep}_{b}_{i}_{h}",
                                  tag="rc")
                    nc.vector.reciprocal(
                        rc[:], asb[DH:DH + 1, h * SCHUNK:(h + 1) * SCHUNK])
                    zbs = zbp.tile([DH, SCHUNK], F32, name=f"zbs_{rep}_{b}_{i}_{h}",
                                   tag="zb")
                    nc.gpsimd.partition_broadcast(zbs[:], rc[:])
                    nc.vector.tensor_mul(
                        at_all[h * DH:(h + 1) * DH, :],
                        asb[0:DH, h * SCHUNK:(h + 1) * SCHUNK], zbs[:])

                # stage this chunk's attention output for the AllToAll:
                # chunk i covers row-ranges 2i and 2i+1 of batch b
                for half in range(2):
                    r0 = (2 * i + half) * DC
                    nc.sync.dma_start(
                        a_dr[b][r0:r0 + DC, :],
                        at_all[:, half * RPB:(half + 1) * RPB])

            def do_a2a(b):
                if collective:
                    nc.gpsimd.collective_compute(
                        "AllToAll",
                        mybir.AluOpType.bypass,
                        replica_groups=[list(range(N_CORES))],
                        ins=[a_dr[b][:]],
                        outs=[g_dr[b][:]],
                    )

            def outproj_parts(b):
                rep = rep_box[0]
                # load gathered A^T [1024 feats, 256 rows] as k-tile blocks
                gsb = gsp.tile([128, KT * RPB], BF, name=f"gsb_{rep}_{b}",
                               tag="gsb")
                nc.sync.dma_start(
                    gsb[:].rearrange("p (k c) -> p k c", k=KT),
                    g_dr[b][:].rearrange("(k p) c -> p k c", p=128))
                ops = {}

                def mm(rt, n, ks):
                    if n == 0 and ks[0] == 0:
                        ops[rt] = ps.tile([128, 2 * SCHUNK], F32,
                                          name=f"op_{rep}_{b}_{rt}",
                                          tag="pair", bufs=3)
                    for k in ks:
                        nc.tensor.matmul(
                            ops[rt][:, n * SCHUNK:(n + 1) * SCHUNK],
                            gsb[:, RPB * k + TT * rt:RPB * k + TT * (rt + 1)],
                            wo_sb[:, D * k + SCHUNK * n:
                                  D * k + SCHUNK * (n + 1)],
                            start=(k == 0), stop=(k == KT - 1))

                def fin(rt):
                    osb = outsbp.tile([128, D], F32, name=f"osb_{rep}_{b}_{rt}",
                                      tag="osb")
                    nc.vector.tensor_copy(osb[:], ops.pop(rt)[:])
                    nc.sync.dma_start(
                        out_d[(2 * b + rt) * TT:(2 * b + rt + 1) * TT, :],
                        osb[:])

                parts = []
                for rt in range(2):
                    for n in range(2):
                        parts.append(lambda rt=rt, n=n: mm(rt, n, [0, 1, 2, 3]))
                        parts.append(lambda rt=rt, n=n: mm(rt, n, [4, 5, 6, 7]))
                    parts.append(lambda rt=rt: fin(rt))
                return parts

            # Software pipeline. Per step: prefetch x for chunk ci+1,
            # attention for chunk ci-1, projections for chunk ci. The batch-0
            # AllToAll is issued as soon as attn(0,3) has staged its output;
            # its output projection weaves into attn(1,0).
            chunks = [(b, i) for b in range(B) for i in range(NCH)]
            for rep_i in range(repeat):
                rep_box[0] = rep_i
                pending = {}
                for ci in range(len(chunks) + 1):
                    if ci == 0:
                        prefetch_x(*chunks[0])
                        prefetch_x(*chunks[1])
                    elif ci + 1 < len(chunks):
                        prefetch_x(*chunks[ci + 1])
                    if ci == 1:
                        load_wo()
                    weave = []
                    if 1 <= ci < len(chunks):
                        weave += proj_parts(*chunks[ci])
                    # output-projection parts come AFTER projection parts and
                    # two chunks after their AllToAll was issued, so the PE
                    # queue never head-of-line blocks on the exchange
                    weave += pending.pop(ci, [])
                    if ci >= 1 and stage >= 2:
                        b_prev, i_prev = chunks[ci - 1]
                        do_attn(b_prev, i_prev, weave=weave)
                        if stage >= 3 and i_prev == NCH - 1:
                            do_a2a(b_prev)
                            pending[ci + 2] = outproj_parts(b_prev)
                    else:
                        for part in weave:
                            part()
                    if ci == 0:
                        do_proj(*chunks[0])
                for parts in pending.values():
                    for part in parts:
                        part()

    nc.compile()
    return nc


_NC_CACHE = {}


def _get_nc():
    if "nc" not in _NC_CACHE:
        _NC_CACHE["nc"] = build_nc()
    return _NC_CACHE["nc"]


def _host_inputs(x, Wq, Wk, Wv, Wo):
    xt = np.ascontiguousarray(
        np.asarray(x, dtype=np.float32).reshape(ROWS, D).T).astype(BF16)
    tri = (np.arange(TT)[:, None] <= np.arange(TT)[None, :])
    mask2 = np.concatenate([tri, tri], axis=1).astype(BF16)
    ident = np.eye(128, dtype=BF16)
    sel2 = np.zeros((128, 2), dtype=np.float32)
    sel2[:DH, 0] = 1.0
    sel2[DH:2 * DH, 1] = 1.0
    sel2t = np.ascontiguousarray(sel2.T).astype(BF16)
    sel2 = sel2.astype(BF16)

    in_maps = []
    for c in range(N_CORES):
        cs = c * DC
        in_maps.append({
            "xt": xt,
            "wq": np.ascontiguousarray(np.asarray(Wq, dtype=np.float32)[:, cs:cs + DC]).astype(BF16),
            "wk": np.ascontiguousarray(np.asarray(Wk, dtype=np.float32)[:, cs:cs + DC]).astype(BF16),
            "wv": np.ascontiguousarray(np.asarray(Wv, dtype=np.float32)[:, cs:cs + DC]).astype(BF16),
            "wo": np.asarray(Wo, dtype=np.float32).astype(BF16),
            "mask2": mask2,
            "ident": ident,
            "sel2": sel2,
            "sel2t": sel2t,
        })
    return in_maps


def kernel(x, Wq, Wk, Wv, Wo, mask):
    x = np.asarray(x, dtype=np.float32)
    nc = _get_nc()
    in_maps = _host_inputs(x, np.asarray(Wq), np.asarray(Wk),
                           np.asarray(Wv), np.asarray(Wo))
    res = run_bass_kernel_spmd(nc, in_maps, list(range(N_CORES)))
    full = np.empty((ROWS, D), dtype=np.float32)
    for c in range(N_CORES):
        o = res.results[c]["out"]
        for b in range(B):
            r0 = b * S + c * RPB
            full[r0:r0 + RPB] = o[b * RPB:(b + 1) * RPB]
    return full.reshape(B, S, D)


if __name__ == "__main__":
    nc = build_nc()
    print("kernel built and compiled OK")


# revision 38
# speedup vs baseline: 4.8977x; 3.4066x over previous
"""Trainium2 Bass kernel for nn_Attention_47545287967487.

Causal multi-head attention (B=2, S=2048, D=1024, H=16, DH=64) with QK
RMS-norm, distributed over 8 NeuronCores.

Distribution: head tensor-parallel for the QKV projections and attention
(each core owns 2 heads = a 128-column slice of Wq/Wk/Wv, computing the
full 4096-row sequence), then per-batch AllToAlls redistribute the bf16
attention outputs so each core owns a 256-row slice per batch and runs
the output projection locally with the full Wo. This moves ~1MB/core
over the fabric instead of ReduceScattering a 16MB fp32 partial, and
the final out write is 2MB instead of 16MB. The batch-0 AllToAll
overlaps batch-1 attention; its output projection weaves into the
batch-1 pipeline.

Numerics: x/W/QK^T/PV run in bf16 (fp32 PSUM accumulation), softmax in
fp32->bf16. Scores are bounded (|q.k|/8 <= 8 after RMS-norm) so softmax
skips the max-subtraction pass; a constant -2.25 bias inside the exp
keeps headroom for an fp8 probability variant (cancels in the
normalization).

Engine plan per core:
 - PE: x@W projections (bf16, K-tiled), QK^T with the two heads packed
   into array row-groups (tile_position), PV as [v|1]^T @ P so the
   softmax denominator is a free 65th output row, the local output
   projection, selector matmuls for the q+k sum-of-squares rows and the
   rstd broadcast, and 128x128 transposes of v into [t,d] layout.
 - ACT: one exp per (chunk, key-tile) covering BOTH heads via a
   [128,2,npx] access pattern over a 2-bank PSUM pair tile; ONE
   Square / Ln / Exp per chunk for the q+k rstd chain (q and k side by
   side on partitions 0:2 of a [2,1024] tile). The 1/sqrt(DH) score
   scale folds into the q normalize multiply.
 - DVE: PSUM->SBUF casts/copies, q/k normalize multiplies, causal mask
   multiplies (only the 128-col diagonal block), softmax denominator
   reciprocals and attention-output normalize multiplies.
 - Pool/GPSIMD: denominator partition broadcasts and the AllToAll
   collectives (everything else is kept off Pool so a collective's
   engine-occupancy can't stall the compute pipeline; att accumulators
   are copied PSUM->SBUF so their PSUM slots recycle without waiting on
   the normalize chain).

PSUM discipline: exactly 8 banks = one pool with a [128,1024] fp32
"pair" tag (bufs=3; holds q+k projection pairs, v projections + v
transposes, the sumsq/broadcast scratch, score pairs, and output-
projection pairs) plus two [65,512] attention accumulators.

kernel(**inputs) takes the FULL unsharded inputs and returns the FULL
[2, 2048, 1024] float32 output.
"""

import math
import numpy as np

import concourse.bacc as bacc
import concourse.mybir as mybir
from concourse import tile
from concourse.bass_utils import run_bass_kernel_spmd

import ml_dtypes

BF16 = ml_dtypes.bfloat16

# Problem shape (hardcoded per the harness contract).
B, S, D, DH = 2, 2048, 1024, 64
H = D // DH
N_CORES = 8
HEADS_PER_CORE = H // N_CORES          # 2
DC = HEADS_PER_CORE * DH               # 128 feature columns per core
EPS = 1e-6

SCHUNK = 512                            # s-chunk width
TT = 128                                # t-tile width
KT = D // 128                           # 8 contraction tiles
NCH = S // SCHUNK                       # 4 s-chunks per batch
ROWS = B * S                            # 4096
RPB = S // N_CORES                      # 256 output rows per core per batch
EXP_BIAS = -2.25                        # softmax headroom shift (cancels)
QSCALE = 1.0 / (DH ** 0.5)              # folded into the q normalize

F32 = mybir.dt.float32
F32R = mybir.dt.float32r
BF = mybir.dt.bfloat16
FP8 = mybir.dt.float8e4
FP8NP = mybir.dt.np(FP8)

# All ACT functions this kernel uses (Square, Ln, Exp, Copy) live in the
# 'natural_log_exp_and_others' table. The default table chooser picks the
# first table containing each function, which thrashes between the exp and
# ln tables (~1.3us per reload, dozens of reloads). Pin the chooser to the
# one table that covers everything by emptying the others (positions are
# preserved so act_func_set_id still indexes act_info.json correctly).
_PINNED_ACT_TABLE = "natural_log_exp_and_others"
_orig_get_act_tables = bacc.get_activation_tables


def _pinned_act_tables(arch):
    tables = _orig_get_act_tables(arch)
    return {
        name: (funcs if name == _PINNED_ACT_TABLE else set())
        for name, funcs in tables.items()
    }


bacc.get_activation_tables = _pinned_act_tables


def build_nc(collective=True, stage=3, repeat=1):
    nc = bacc.Bacc("TRN2", target_bir_lowering=False)

    xt_d = nc.dram_tensor("xt", [D, ROWS], BF, kind="ExternalInput")
    wq_d = nc.dram_tensor("wq", [D, DC], BF, kind="ExternalInput")
    wk_d = nc.dram_tensor("wk", [D, DC], BF, kind="ExternalInput")
    wv_d = nc.dram_tensor("wv", [D, DC], BF, kind="ExternalInput")
    wo_d = nc.dram_tensor("wo", [D, D], BF, kind="ExternalInput")
    mask2_d = nc.dram_tensor("mask2", [TT, 2 * TT], BF, kind="ExternalInput")
    ident_d = nc.dram_tensor("ident", [128, 128], BF, kind="ExternalInput")
    sel2_d = nc.dram_tensor("sel2", [128, 2], BF, kind="ExternalInput")
    sel2t_d = nc.dram_tensor("sel2t", [2, 128], BF, kind="ExternalInput")
    out_d = nc.dram_tensor("out", [2 * RPB, D], F32, kind="ExternalOutput")

    from contextlib import ExitStack
    with tile.TileContext(nc) as tc:
        with ExitStack() as ctx:
            consts = ctx.enter_context(tc.tile_pool(name="consts", bufs=1))
            wpool = ctx.enter_context(tc.tile_pool(name="wpool", bufs=1))
            persist = ctx.enter_context(tc.tile_pool(name="persist", bufs=1))
            xcp = ctx.enter_context(tc.tile_pool(name="xc", bufs=2))
            sqp = ctx.enter_context(tc.tile_pool(name="sqp", bufs=2))
            stdp = ctx.enter_context(tc.tile_pool(name="stdp", bufs=2))
            bcp = ctx.enter_context(tc.tile_pool(name="bcp", bufs=2))
            vtp = ctx.enter_context(tc.tile_pool(name="vtp", bufs=2))
            vaugp = ctx.enter_context(tc.tile_pool(name="vaugp", bufs=36))
            pp = ctx.enter_context(tc.tile_pool(name="pp", bufs=4))
            attsbp = ctx.enter_context(tc.tile_pool(name="attsb", bufs=4))
            zbp = ctx.enter_context(tc.tile_pool(name="zbp", bufs=4))
            rcp = ctx.enter_context(tc.tile_pool(name="rcp", bufs=4))
            attallp = ctx.enter_context(tc.tile_pool(name="attall", bufs=3))
            gsp = ctx.enter_context(tc.tile_pool(name="gsp", bufs=2))
            outsbp = ctx.enter_context(tc.tile_pool(name="outsb", bufs=2))
            ps = ctx.enter_context(tc.tile_pool(name="ps", bufs=1, space="PSUM"))
            dram = ctx.enter_context(tc.tile_pool(name="dram", bufs=1, space="DRAM"))

            # ---- weights first (gate the first projections), then consts,
            # wo last (only needed at the first output projection) ----
            # One merged DMA per weight tensor: SBUF [128, KT*DC] where
            # free-column block k holds DRAM rows [128k, 128k+128) (the
            # k-th contraction tile), so lhsT slices stay [K=128, M=DC].
            w_sb = {}
            for wname, wd in (("q", wq_d), ("k", wk_d), ("v", wv_d)):
                t = wpool.tile([128, KT * DC], BF, name=f"w{wname}")
                nc.sync.dma_start(
                    t[:].rearrange("p (k c) -> p k c", k=KT),
                    wd[:].rearrange("(k p) c -> p k c", p=128))
                for k in range(KT):
                    w_sb[(wname, k)] = t[:, k * DC:(k + 1) * DC]

            sel2_sb = consts.tile([128, 2], BF, name="sel2_sb")
            nc.sync.dma_start(sel2_sb[:], sel2_d[:])
            sel2t_sb = consts.tile([2, 128], BF, name="sel2t_sb")
            nc.sync.dma_start(sel2t_sb[:], sel2t_d[:])
            ident_sb = consts.tile([128, 128], BF, name="ident_sb")
            nc.sync.dma_start(ident_sb[:], ident_d[:])
            mask2_sb = consts.tile([TT, 2 * TT], BF, name="mask2_sb")
            nc.sync.dma_start(mask2_sb[:], mask2_d[:])
            eps_sb = consts.tile([2, 1], F32, name="eps_sb")
            nc.vector.memset(eps_sb[:], EPS)
            zero_sb = consts.tile([2, 1], F32, name="zero_sb")
            nc.vector.memset(zero_sb[:], 0.0)
            ebias_sb = consts.tile([128, 1], F32, name="ebias_sb")
            nc.vector.memset(ebias_sb[:], EXP_BIAS)
            # wo is only needed by the first output projection (~half-way in);
            # its 2MB DMA is deferred into the pipeline so it doesn't delay
            # the first x-chunk prefetches behind it in the queue.
            wo_sb = wpool.tile([128, KT * D], BF, name="wo_sb")
            wo_loaded = [False]

            def load_wo():
                if not wo_loaded[0]:
                    wo_loaded[0] = True
                    nc.sync.dma_start(
                        wo_sb[:].rearrange("p (k c) -> p k c", k=KT),
                        wo_d[:].rearrange("(k p) c -> p k c", p=128))

            # DRAM staging for the per-batch AllToAll of attention outputs.
            # a[b] row-block j = this core's 128 features for row-range
            # [RPB*j, RPB*j+RPB) of batch b; after AllToAll g[b] row-block s
            # = core s's features for THIS core's row range.
            a_dr = [dram.tile([N_CORES * DC, RPB], BF, name=f"a_dr{b}")
                    for b in range(B)]
            if collective:
                g_dr = [dram.tile([N_CORES * DC, RPB], BF, name=f"g_dr{b}")
                        for b in range(B)]
            else:
                g_dr = a_dr  # collective-free variant for TimelineSim

            # per-chunk normalized q/k bf16, feature-major.
            qts = {}    # (b, i) -> [DC, SCHUNK] bf16, q * rstd_q / sqrt(DH)
            kts = {}    # (b, i) -> [DC, SCHUNK] bf16, k * rstd_k
            vaug = {}   # (b, j) -> [128, 2*(DH+1)] bf16: [v|1] per head
            pqks = {}   # (b, i) -> in-flight q|k projection PSUM pair

            xcs = {}
            rep_box = [0]

            def prefetch_x(b, i):
                rep = rep_box[0]
                col0 = b * S + i * SCHUNK
                xc = xcp.tile([128, KT * SCHUNK], BF, name=f"x_{rep}_{b}_{i}",
                              tag="xc")
                # one DMA per k-tile: cheaper first-tile latency, and the
                # first projection matmul can start before the rest land
                for k in range(KT):
                    nc.sync.dma_start(
                        xc[:, k * SCHUNK:(k + 1) * SCHUNK],
                        xt_d[k * 128:(k + 1) * 128, col0:col0 + SCHUNK])
                xcs[(b, i)] = xc

            def proj_qk_mm(b, i, xch, half, ks):
                rep = rep_box[0]
                if half == 0 and ks[0] == 0:
                    pqks[(b, i)] = ps.tile(
                        [128, 2 * SCHUNK], F32, name=f"pqk_{rep}_{b}_{i}",
                        tag="pair", bufs=3)
                pqk = pqks[(b, i)]
                wname = "qk"[half]
                for k in ks:
                    nc.tensor.matmul(
                        pqk[:, half * SCHUNK:(half + 1) * SCHUNK],
                        w_sb[(wname, k)][:], xch[k][:], start=(k == 0),
                        stop=(k == KT - 1))

            def proj_qk_norm(b, i):
                rep = rep_box[0]
                # raw q|k off PSUM immediately (frees the pair slot); the
                # whole normalize chain then runs in bf16 (2x DVE/ACT).
                pqk = pqks.pop((b, i))
                qkr = sqp.tile([128, 2 * SCHUNK], BF, name=f"qkr_{rep}_{b}_{i}",
                               tag="qkr")
                nc.vector.tensor_copy(qkr[:], pqk[:])
                sq = sqp.tile([128, 2 * SCHUNK], BF, name=f"sq_{rep}_{b}_{i}",
                              tag="sq")
                nc.scalar.activation(sq[:], qkr[:],
                                     mybir.ActivationFunctionType.Square)

                # sum-of-squares per 64-row head group: q heads at cols
                # [0:512] of rows 0:2, k heads at cols [512:1024].
                ssbc = ps.tile([128, 2 * SCHUNK], F32, name=f"ssbc_{rep}_{b}_{i}",
                               tag="pair", bufs=3)
                nc.tensor.matmul(ssbc[0:2, 0:SCHUNK], sel2_sb[:],
                                 sq[:, 0:SCHUNK], start=True, stop=True)
                nc.tensor.matmul(ssbc[0:2, SCHUNK:2 * SCHUNK], sel2_sb[:],
                                 sq[:, SCHUNK:2 * SCHUNK], start=True,
                                 stop=True)
                lm = stdp.tile([2, 2 * SCHUNK], F32, name=f"lm_{rep}_{b}_{i}",
                               tag="lm")
                nc.scalar.activation(lm[:], ssbc[0:2, :],
                                     mybir.ActivationFunctionType.Ln,
                                     scale=1.0 / DH, bias=eps_sb[:])
                rstd = stdp.tile([2, 2 * SCHUNK], BF, name=f"rstd_{rep}_{b}_{i}",
                                 tag="rstd")
                nc.scalar.activation(rstd[:], lm[:],
                                     mybir.ActivationFunctionType.Exp,
                                     scale=-0.5, bias=zero_sb[:])

                # broadcast both rstds over the head 64-row groups (WAR with
                # the sumsq rows is tracked; Ln has consumed them by then)
                for half in range(2):
                    nc.tensor.matmul(
                        ssbc[:, half * SCHUNK:(half + 1) * SCHUNK],
                        sel2t_sb[:], rstd[:, half * SCHUNK:(half + 1) * SCHUNK],
                        start=True, stop=True)
                bcs = bcp.tile([DC, 2 * SCHUNK], BF, name=f"bcs_{rep}_{b}_{i}",
                               tag="bc")
                nc.vector.tensor_copy(bcs[:], ssbc[:])

                qtile = persist.tile([DC, SCHUNK], BF, name=f"qt_{rep}_{b}_{i}",
                                     tag="qt", bufs=3)
                qts[(b, i)] = qtile
                nc.vector.scalar_tensor_tensor(
                    qtile[:], qkr[:, 0:SCHUNK], QSCALE, bcs[:, 0:SCHUNK],
                    mybir.AluOpType.mult, mybir.AluOpType.mult)
                ktile = persist.tile([DC, SCHUNK], BF, name=f"kt_{rep}_{b}_{i}",
                                     tag="kt", bufs=8)
                kts[(b, i)] = ktile
                nc.vector.tensor_mul(ktile[:], qkr[:, SCHUNK:2 * SCHUNK],
                                     bcs[:, SCHUNK:2 * SCHUNK])

            def proj_v_mm(b, i, xch, ks):
                rep = rep_box[0]
                if ks[0] == 0:
                    pqks[(b, i, "v")] = ps.tile(
                        [128, 2 * SCHUNK], F32, name=f"pv_{rep}_{b}_{i}",
                        tag="pair", bufs=3)
                psv = pqks[(b, i, "v")]
                for k in ks:
                    nc.tensor.matmul(psv[:, 0:SCHUNK], w_sb[("v", k)][:],
                                     xch[k][:], start=(k == 0),
                                     stop=(k == KT - 1))

            def proj_v_fin(b, i):
                rep = rep_box[0]
                psv = pqks.pop((b, i, "v"))
                vt = vtp.tile([DC, SCHUNK], BF, name=f"vt_{rep}_{b}_{i}",
                              tag="vt")
                nc.vector.tensor_copy(vt[:], psv[:, 0:SCHUNK])
                # transposes reuse the (dead) second bank of the psv slot
                for u in range(SCHUNK // TT):
                    tpb = psv[:, SCHUNK + 64 * u:SCHUNK + 64 * (u + 1)].bitcast(BF)
                    nc.tensor.transpose(tpb[:], vt[:, u * 128:(u + 1) * 128],
                                        ident_sb[:])
                for u in range(SCHUNK // TT):
                    j = i * (SCHUNK // TT) + u
                    tpb = psv[:, SCHUNK + 64 * u:SCHUNK + 64 * (u + 1)].bitcast(BF)
                    va = vaugp.tile([128, 2 * (DH + 1)], BF,
                                    name=f"va_{rep}_{b}_{j}", tag="vaug")
                    nc.vector.tensor_copy(
                        va[:].rearrange("p (g d) -> p g d", g=2)[:, :, 0:DH],
                        tpb[:].rearrange("p (g d) -> p g d", g=2))
                    nc.vector.memset(
                        va[:].rearrange("p (g d) -> p g d", g=2)[:, :, DH:DH + 1],
                        1.0)
                    vaug[(b, j)] = va

            def proj_parts(b, i):
                xc = xcs.pop((b, i))
                xch = [xc[:, k * SCHUNK:(k + 1) * SCHUNK] for k in range(KT)]
                return [
                    lambda: proj_qk_mm(b, i, xch, 0, [0, 1, 2, 3]),
                    lambda: proj_qk_mm(b, i, xch, 0, [4, 5, 6, 7]),
                    lambda: proj_qk_mm(b, i, xch, 1, [0, 1, 2, 3]),
                    lambda: proj_qk_mm(b, i, xch, 1, [4, 5, 6, 7]),
                    lambda: proj_qk_norm(b, i),
                    lambda: proj_v_mm(b, i, xch, [0, 1, 2, 3]),
                    lambda: proj_v_mm(b, i, xch, [4, 5, 6, 7]),
                    lambda: proj_v_fin(b, i),
                ]

            def do_proj(b, i):
                for part in proj_parts(b, i):
                    part()

            def do_attn(b, i, weave=None):
                rep = rep_box[0]
                att = [ps.tile([DH + 1, SCHUNK], F32,
                               name=f"att_{rep}_{b}_{i}_{h}", tag=f"att{h}",
                               bufs=1)
                       for h in range(HEADS_PER_CORE)]
                n_t = 4 * i + 4
                parts = list(weave) if weave else []
                npop = len(parts)
                psbs = {}

                def pv(j):
                    # PV for tile j, one software-pipeline stage behind the
                    # exp so the PE never waits on the current tile's exp
                    offj = max(0, TT * (j - 4 * i))
                    pj = psbs.pop(j)
                    for h in range(HEADS_PER_CORE):
                        nc.tensor.matmul(
                            att[h][:, offj:SCHUNK],
                            vaug[(b, j)][:, h * (DH + 1):(h + 1) * (DH + 1)],
                            pj[:, SCHUNK * h + offj:SCHUNK * (h + 1)],
                            start=(j == 0), stop=(j == n_t - 1),
                        )

                for j in range(n_t):
                    off = max(0, TT * (j - 4 * i))
                    npx = SCHUNK - off
                    jc, ju = j // 4, j % 4
                    # both heads' scores in one 2-bank pair tile: head h at
                    # cols [512h+off, 512h+512)
                    pt = ps.tile([128, 2 * SCHUNK], F32,
                                 name=f"ptile_{rep}_{b}_{i}_{j}", tag="pair", bufs=3)
                    for h in range(HEADS_PER_CORE):
                        nc.tensor.matmul(
                            pt[:, SCHUNK * h + off:SCHUNK * (h + 1)],
                            kts[(b, jc)][h * DH:(h + 1) * DH,
                                         ju * TT:(ju + 1) * TT],
                            qts[(b, i)][h * DH:(h + 1) * DH, off:SCHUNK],
                            start=True, stop=True,
                            tile_position=(h * DH, 0),
                        )
                    # one exp covers both heads via the [128, 2, npx] view
                    psb = pp.tile([128, 2 * SCHUNK], BF,
                                  name=f"p_{rep}_{b}_{i}_{j}", tag="p")
                    psbs[j] = psb
                    ptv = pt[:].rearrange("p (h c) -> p h c", h=2)
                    psv = psb[:].rearrange("p (h c) -> p h c", h=2)
                    nc.scalar.activation(
                        psv[:, :, off:SCHUNK], ptv[:, :, off:SCHUNK],
                        mybir.ActivationFunctionType.Exp, bias=ebias_sb[:])
                    if j >= 4 * i:
                        # causal mask: only the first 128 columns of the
                        # diagonal block differ from all-ones
                        nc.vector.tensor_mul(
                            psv[:, :, off:off + TT], psv[:, :, off:off + TT],
                            mask2_sb[:].rearrange("p (h c) -> p h c", h=2))
                    # weave projection/output-projection work between this
                    # tile's exp and the PREVIOUS tile's PV, paced evenly
                    # across j-tiles so the PE stays fed while ACT runs
                    want = (npop * (j + 1)) // n_t
                    while npop - len(parts) < want:
                        parts.pop(0)()
                    if j > 0:
                        pv(j - 1)
                pv(n_t - 1)

                for part in parts:
                    part()

                # copy accumulators off PSUM immediately (releases the att
                # banks for the next chunk without waiting on the normalize
                # chain, which can stall behind a collective on Pool)
                asb = attsbp.tile([DH + 1, 2 * SCHUNK], F32,
                                  name=f"asb_{rep}_{b}_{i}", tag="asb")
                for h in range(HEADS_PER_CORE):
                    nc.vector.tensor_copy(
                        asb[:, h * SCHUNK:(h + 1) * SCHUNK], att[h][:])

                # ---- normalize by softmax denominator ----
                at_all = attallp.tile([DC, SCHUNK], BF,
                                      name=f"atall_{rep}_{b}_{i}", tag="attall")
                for h in range(HEADS_PER_CORE):
                    rc = rcp.tile([1, SCHUNK], F32, name=f"rc_{rep}_{b}_{i}_{h}",
                                  tag="rc")
                    nc.vector.reciprocal(
                        rc[:], asb[DH:DH + 1, h * SCHUNK:(h + 1) * SCHUNK])
                    zbs = zbp.tile([DH, SCHUNK], F32, name=f"zbs_{rep}_{b}_{i}_{h}",
                                   tag="zb")
                    nc.gpsimd.partition_broadcast(zbs[:], rc[:])
                    nc.vector.tensor_mul(
                        at_all[h * DH:(h + 1) * DH, :],
                        asb[0:DH, h * SCHUNK:(h + 1) * SCHUNK], zbs[:])

                # stage this chunk's attention output for the AllToAll:
                # chunk i covers row-ranges 2i and 2i+1 of batch b
                for half in range(2):
                    r0 = (2 * i + half) * DC
                    nc.sync.dma_start(
                        a_dr[b][r0:r0 + DC, :],
                        at_all[:, half * RPB:(half + 1) * RPB])

            def do_a2a(b):
                if collective:
                    nc.gpsimd.collective_compute(
                        "AllToAll",
                        mybir.AluOpType.bypass,
                        replica_groups=[list(range(N_CORES))],
                        ins=[a_dr[b][:]],
                        outs=[g_dr[b][:]],
                    )

            def outproj_parts(b):
                rep = rep_box[0]
                # load gathered A^T [1024 feats, 256 rows] as k-tile blocks
                gsb = gsp.tile([128, KT * RPB], BF, name=f"gsb_{rep}_{b}",
                               tag="gsb")
                nc.sync.dma_start(
                    gsb[:].rearrange("p (k c) -> p k c", k=KT),
                    g_dr[b][:].rearrange("(k p) c -> p k c", p=128))
                ops = {}

                def mm(rt, n, ks):
                    if n == 0 and ks[0] == 0:
                        ops[rt] = ps.tile([128, 2 * SCHUNK], F32,
                                          name=f"op_{rep}_{b}_{rt}",
                                          tag="pair", bufs=3)
                    for k in ks:
                        nc.tensor.matmul(
                            ops[rt][:, n * SCHUNK:(n + 1) * SCHUNK],
                            gsb[:, RPB * k + TT * rt:RPB * k + TT * (rt + 1)],
                            wo_sb[:, D * k + SCHUNK * n:
                                  D * k + SCHUNK * (n + 1)],
                            start=(k == 0), stop=(k == KT - 1))

                def fin(rt):
                    osb = outsbp.tile([128, D], F32, name=f"osb_{rep}_{b}_{rt}",
                                      tag="osb")
                    nc.vector.tensor_copy(osb[:], ops.pop(rt)[:])
                    nc.sync.dma_start(
                        out_d[(2 * b + rt) * TT:(2 * b + rt + 1) * TT, :],
                        osb[:])

                parts = []
                for rt in range(2):
                    for n in range(2):
                        parts.append(lambda rt=rt, n=n: mm(rt, n, [0, 1, 2, 3]))
                        parts.append(lambda rt=rt, n=n: mm(rt, n, [4, 5, 6, 7]))
                    parts.append(lambda rt=rt: fin(rt))
                return parts

            # Software pipeline. Per step: prefetch x for chunk ci+1,
            # attention for chunk ci-1, projections for chunk ci. The batch-0
            # AllToAll is issued as soon as attn(0,3) has staged its output;
            # its output projection weaves into attn(1,0).
            chunks = [(b, i) for b in range(B) for i in range(NCH)]
            pending = {}
            for rep_i in range(repeat):
                rep_box[0] = rep_i
                gbase = rep_i * (len(chunks) + 1)
                for ci in range(len(chunks) + 1):
                    if ci == 0:
                        if rep_i == 0:
                            prefetch_x(*chunks[0])
                            prefetch_x(*chunks[1])
                    elif ci + 1 < len(chunks):
                        prefetch_x(*chunks[ci + 1])
                    elif ci == len(chunks) and rep_i + 1 < repeat:
                        # prefetch the next repeat's first chunks before its
                        # boundary so its projections start immediately
                        prefetch_x(*chunks[0])
                        prefetch_x(*chunks[1])
                    if ci == 1:
                        load_wo()
                    weave = []
                    if 1 <= ci < len(chunks):
                        weave += proj_parts(*chunks[ci])
                    # output-projection parts come AFTER projection parts and
                    # two chunks after their AllToAll was issued (crossing the
                    # repeat boundary if needed), so the PE queue never
                    # head-of-line blocks on the exchange
                    weave += pending.pop(gbase + ci, [])
                    if ci >= 1 and stage >= 2:
                        b_prev, i_prev = chunks[ci - 1]
                        do_attn(b_prev, i_prev, weave=weave)
                        if stage >= 3 and i_prev == NCH - 1:
                            do_a2a(b_prev)
                            pending[gbase + ci + 2] = outproj_parts(b_prev)
                    else:
                        for part in weave:
                            part()
                    if ci == 0:
                        do_proj(*chunks[0])
            for parts in pending.values():
                for part in parts:
                    part()

    nc.compile()
    return nc


_NC_CACHE = {}


def _get_nc():
    if "nc" not in _NC_CACHE:
        _NC_CACHE["nc"] = build_nc()
    return _NC_CACHE["nc"]


def _host_inputs(x, Wq, Wk, Wv, Wo):
    xt = np.ascontiguousarray(
        np.asarray(x, dtype=np.float32).reshape(ROWS, D).T).astype(BF16)
    tri = (np.arange(TT)[:, None] <= np.arange(TT)[None, :])
    mask2 = np.concatenate([tri, tri], axis=1).astype(BF16)
    ident = np.eye(128, dtype=BF16)
    sel2 = np.zeros((128, 2), dtype=np.float32)
    sel2[:DH, 0] = 1.0
    sel2[DH:2 * DH, 1] = 1.0
    sel2t = np.ascontiguousarray(sel2.T).astype(BF16)
    sel2 = sel2.astype(BF16)

    in_maps = []
    for c in range(N_CORES):
        cs = c * DC
        in_maps.append({
            "xt": xt,
            "wq": np.ascontiguousarray(np.asarray(Wq, dtype=np.float32)[:, cs:cs + DC]).astype(BF16),
            "wk": np.ascontiguousarray(np.asarray(Wk, dtype=np.float32)[:, cs:cs + DC]).astype(BF16),
            "wv": np.ascontiguousarray(np.asarray(Wv, dtype=np.float32)[:, cs:cs + DC]).astype(BF16),
            "wo": np.asarray(Wo, dtype=np.float32).astype(BF16),
            "mask2": mask2,
            "ident": ident,
            "sel2": sel2,
            "sel2t": sel2t,
        })
    return in_maps


def kernel(x, Wq, Wk, Wv, Wo, mask):
    x = np.asarray(x, dtype=np.float32)
    nc = _get_nc()
    in_maps = _host_inputs(x, np.asarray(Wq), np.asarray(Wk),
                           np.asarray(Wv), np.asarray(Wo))
    res = run_bass_kernel_spmd(nc, in_maps, list(range(N_CORES)))
    full = np.empty((ROWS, D), dtype=np.float32)
    for c in range(N_CORES):
        o = res.results[c]["out"]
        for b in range(B):
            r0 = b * S + c * RPB
            full[r0:r0 + RPB] = o[b * RPB:(b + 1) * RPB]
    return full.reshape(B, S, D)


if __name__ == "__main__":
    nc = build_nc()
    print("kernel built and compiled OK")
